# revision 30
# baseline (speedup 1.0000x reference)
"""Trainium2 Bass kernel for nn_MixedAttnHeadEmbed (mixed-head-config attention).

Math (per batch b):
  Two attention configs share q_m/k_m/v_m [B,T,2048]:
    A: h=8  heads, d_max=256, mixing e in {1024,2048} -> d in {128,256}, weights w0,w1
    B: h=16 heads, d_max=128, mixing e in {1024,2048} -> d in {64,128},  weights w2,w3
  Each config: per-head q/k slices are RoPE'd, weight-summed (padded to d_max),
  GQA (8 kv heads), causal softmax attention; outputs of both configs sum.

Sharding: 8 cores = 4 batches x 2 shards. Shard s owns A-heads [4s,4s+4) and
B-heads [8s,8s+8) -> both write output columns [1024s, 1024s+1024) which are
summed on device; per-core output is the transposed block outT [1024, T].

Device layout: scores computed transposed (sT[k,q], k on partitions) so the
softmax'd weights feed the y^T matmul with no on-chip transposes; softmax is
max-free (scores are provably < 2 for this problem family; exp is safe in
fp32) with the denominator from an all-ones stationary matmul.
"""

import math
from contextlib import ExitStack
from dataclasses import dataclass

import numpy as np

import concourse.bass as bass
import concourse.mybir as mybir
import concourse.tile as tile
from concourse import bacc

F32 = mybir.dt.float32
NEG = -1e9
P = 128


@dataclass(frozen=True)
class KCfg:
    T: int = 1024       # sequence length
    NA: int = 4         # config-A heads per core (d_max=256)
    NB: int = 8         # config-B heads per core (d_max=128); must be 2*NA
    REG: int = 512      # psum region width (<=512)

    @property
    def TK(self):
        return self.T // P

    @property
    def NREG(self):
        return self.T // self.REG

    @property
    def NKVB(self):
        return self.NB // 2

    @property
    def ROWS(self):
        return self.NA * 256  # == NB * 128 output rows per core


FULL = KCfg()
SHIFT_MODE = "copy"  # how rope's cross-partition-base multiplies are emitted


def _in_specs(cfg: KCfg):
    T = cfg.T
    return {
        "qT1": (cfg.NA * 128, T),
        "qT2": (cfg.NA * 256, T),
        "kTa1": (cfg.NA * 128, T),
        "kTa2": (cfg.NA * 256, T),
        "kTb1": (cfg.NKVB * 64, T),
        "kTb2": (cfg.NKVB * 128, T),
        "va1": (T, cfg.NA * 128),
        "va2": (T, cfg.NA * 256),
        "vb1": (T, cfg.NKVB * 64),
        "vb2": (T, cfg.NKVB * 128),
        "ca1": (128, T), "sa1": (128, T),
        "ca2": (256, T), "sa2": (256, T),
        "cb1": (128, T), "sb1": (128, T),
        "cb2": (128, T), "sb2": (128, T),
        "wvec": (P, 4),
    }


class _EngPick:
    """Static load balancer between DVE (1x cost) and GPSIMD (2x cost)."""

    def __init__(self, nc):
        self.nc = nc
        self.loads = [0.0, 0.0]  # vector, gpsimd

    def __call__(self, units=1.0, dve_only=False):
        # tensor_scalar / scalar_tensor_tensor (TensorScalarPtr) and any
        # PSUM-touching op must go on DVE; gpsimd handles plain TT/copy.
        if dve_only:
            self.loads[0] += units
            return self.nc.vector
        # gpsimd pays 2x per unit
        if self.loads[0] <= self.loads[1] + units:
            self.loads[0] += units
            return self.nc.vector
        self.loads[1] += 2.0 * units
        return self.nc.gpsimd


def build_program(cfg: KCfg = FULL):
    # Bacc (not plain Bass): its compile() runs generate_event_semaphores,
    # which splits multi-wait sync_infos — TRN2 allows 1 wait per instruction.
    nc = bacc.Bacc("TRN2", target_bir_lowering=False)
    T, TK, REG, NREG = cfg.T, cfg.TK, cfg.REG, cfg.NREG
    RPB = REG // P  # k-chunks per region

    D = {}
    for name, shape in _in_specs(cfg).items():
        D[name] = nc.declare_dram_parameter(name, list(shape), F32, isOutput=False)
    outT = nc.declare_dram_parameter("outT", [cfg.ROWS, T], F32, isOutput=True)
    RB = cfg.ROWS // P

    mult, add = mybir.AluOpType.mult, mybir.AluOpType.add

    with ExitStack() as ctx:
        tc = ctx.enter_context(tile.TileContext(nc))
        const = ctx.enter_context(tc.tile_pool(name="const", bufs=1))
        rawp = ctx.enter_context(tc.tile_pool(name="raw", bufs=2))
        mixp = ctx.enter_context(tc.tile_pool(name="mix", bufs=2))
        scr = ctx.enter_context(tc.tile_pool(name="scr", bufs=1))
        ppool = ctx.enter_context(tc.tile_pool(name="pp", bufs=3))
        accp = ctx.enter_context(tc.tile_pool(name="acc", bufs=1))
        normp = ctx.enter_context(tc.tile_pool(name="norm", bufs=1))
        spsum = ctx.enter_context(tc.tile_pool(name="spsum", bufs=2, space="PSUM"))
        ypsum = ctx.enter_context(tc.tile_pool(name="ypsum", bufs=1, space="PSUM"))
        dpsum = ctx.enter_context(tc.tile_pool(name="dpsum", bufs=1, space="PSUM"))

        pick = _EngPick(nc)

        # ---- constants ----
        ones = const.tile([P, P], F32)
        nc.vector.memset(ones, 1.0)
        dmask = const.tile([P, P], F32)
        nc.gpsimd.memset(dmask, 0.0)
        # dmask[k, q] = 0 where q >= k else NEG  (transposed causal diag block)
        nc.gpsimd.affine_select(
            out=dmask, in_=dmask, compare_op=mybir.AluOpType.is_ge,
            fill=NEG, base=0, pattern=[[1, P]], channel_multiplier=-1,
        )
        tabs = {}
        for nm in ("ca1", "sa1", "ca2", "sa2", "cb1", "sb1", "cb2", "sb2"):
            rows = _in_specs(cfg)[nm][0]
            tl = const.tile([P, rows // P, T], F32, name=nm, tag=nm)
            tabs[nm] = tl
            nc.sync.dma_start(out=tl, in_=D[nm].rearrange("(c p) t -> p c t", p=P))
        wv = const.tile([P, 4], F32)
        nc.sync.dma_start(out=wv, in_=D["wvec"][:, :])

        outacc = accp.tile([P, RB, T], F32)

        def xb_view(base, src, units):
            """Return a view of src re-based to partition `base`, via a
            cross-base single-input copy (2-input SBUF ops must share the
            base partition on trn2; 1-input copies may cross)."""
            n = src.shape[0]
            tmp = scr.tile([P, T], F32, tag="xbt", name="xbt")
            view = tmp[base:base + n, :]
            pick(units).tensor_copy(view, src)
            return view

        def xb_mul(dst, base, src, tab, units):
            """dst(@base) = src * tab where src has a DIFFERENT base."""
            if SHIFT_MODE == "stt":
                # (src * 1.0) mult tab via TensorScalarPtr encoding
                pick(units, dve_only=True).scalar_tensor_tensor(
                    out=dst, in0=src, scalar=1.0, in1=tab, op0=mult, op1=mult)
            elif SHIFT_MODE == "copy":
                view = xb_view(base, src, units)
                pick(units).tensor_tensor(dst, view, tab, mult)
            else:
                raise ValueError(SHIFT_MODE)

        def halfmul(dst, src, tab, half):
            """dst[j] = src[sigma(j)] * tab[j] for the rope rotation, where
            sigma swaps halves of size `half` within each 2*half group."""
            n_grp = P // (2 * half)
            for g in range(n_grp):
                b0 = 2 * half * g
                xb_mul(dst[b0:b0 + half, :], b0, src[b0 + half:b0 + 2 * half, :],
                       tab[b0:b0 + half, :], 0.5)
                xb_mul(dst[b0 + half:b0 + 2 * half, :], b0 + half,
                       src[b0:b0 + half, :],
                       tab[b0 + half:b0 + 2 * half, :], 0.5)

        def mix_qk_A(out, x1, x2, c1, s1, c2, s2):
            """out [P,2,T] = rope-mix for a config-A head.
            x1 [P,T] (d=128 slice), x2 [P,2,T] (d=256 slice)."""
            t1 = scr.tile([P, T], F32, tag="t1")
            t2 = scr.tile([P, T], F32, tag="t2")
            # dc0: x2t0*c2_0 + x2t1*s2_0 + x1*c1 + shift64(x1)*s1
            pick().tensor_tensor(out[:, 0, :], x2[:, 0, :], c2[:, 0, :], mult)
            pick().tensor_tensor(t1, x2[:, 1, :], s2[:, 0, :], mult)
            pick().tensor_tensor(out[:, 0, :], out[:, 0, :], t1, add)
            pick().tensor_tensor(t1, x1, c1[:, 0, :], mult)
            halfmul(t2, x1, s1[:, 0, :], 64)
            pick().tensor_tensor(t1, t1, t2, add)
            pick().tensor_tensor(out[:, 0, :], out[:, 0, :], t1, add)
            # dc1: x2t1*c2_1 + x2t0*s2_1
            pick().tensor_tensor(out[:, 1, :], x2[:, 1, :], c2[:, 1, :], mult)
            pick().tensor_tensor(t1, x2[:, 0, :], s2[:, 1, :], mult)
            pick().tensor_tensor(out[:, 1, :], out[:, 1, :], t1, add)

        def xb_add(dst, base, src, units):
            """dst(@base) += src across partition bases."""
            if SHIFT_MODE == "stt":
                pick(units, dve_only=True).scalar_tensor_tensor(
                    out=dst, in0=src, scalar=1.0, in1=dst, op0=mult, op1=add)
            else:
                view = xb_view(base, src, units)
                pick(units).tensor_tensor(dst, dst, view, add)

        def mix_qk_B_pair(out, x1p, x2p, c1, s1, c2, s2):
            """out [P,2,T]: B-head pair. out[:,j,:] for heads (2p+j).
            x2p [P,2,T] (d=128 per head), x1p [P,T] packed pair (d=64 each)."""
            t1 = scr.tile([P, T], F32, tag="t1")
            t2 = scr.tile([P, T], F32, tag="t2")
            for j in range(2):
                pick().tensor_tensor(out[:, j, :], x2p[:, j, :], c2[:, 0, :], mult)
                halfmul(t1, x2p[:, j, :], s2[:, 0, :], 64)
                pick().tensor_tensor(out[:, j, :], out[:, j, :], t1, add)
            # packed d=64 contributions for both heads of the pair
            pick().tensor_tensor(t1, x1p, c1[:, 0, :], mult)
            halfmul(t2, x1p, s1[:, 0, :], 32)
            pick().tensor_tensor(t1, t1, t2, add)
            pick(0.5).tensor_tensor(out[0:64, 0, :], out[0:64, 0, :], t1[0:64, :], add)
            xb_add(out[0:64, 1, :], 0, t1[64:128, :], 0.5)

        def subchunks(c):
            out = []
            for r in range(NREG):
                q0 = max(REG * r, P * c)
                q1 = REG * (r + 1)
                if q1 > q0:
                    out.append((r, q0, q1 - q0))
            return out

        def attn_head(qmixs, kmixs, vmix, blks, is_b):
            """qmixs/kmixs: per-d-chunk [P, T] APs; vmix [P, TK, ndc*P]."""
            ndc = len(qmixs)
            den = dpsum.tile([P, T], F32, tag="den")
            yts = [ypsum.tile([P, T], F32, tag=f"yt{i}", name=f"yt{i}")
                   for i in range(ndc)]
            for c in range(TK):
                for (r, q0, n) in subchunks(c):
                    last_c = min(TK, RPB * (r + 1)) - 1
                    sT = spsum.tile([P, REG], F32, tag="sT")
                    for dc in range(ndc):
                        nc.tensor.matmul(
                            sT[:, :n], kmixs[dc][:, P * c:P * (c + 1)],
                            qmixs[dc][:, q0:q0 + n],
                            start=(dc == 0), stop=(dc == ndc - 1))
                    if q0 == P * c:  # diagonal block gets the causal mask
                        nc.vector.tensor_tensor(sT[:, :P], sT[:, :P], dmask, add)
                    pt = ppool.tile([P, REG], F32, tag="pT")
                    nc.scalar.activation(pt[:, :n], sT[:, :n],
                                         mybir.ActivationFunctionType.Exp)
                    for dc in range(ndc):
                        nc.tensor.matmul(
                            yts[dc][:, q0:q0 + n], vmix[:, c, P * dc:P * (dc + 1)],
                            pt[:, :n], start=(c == 0), stop=(c == last_c))
                    nc.tensor.matmul(den[:, q0:q0 + n], ones, pt[:, :n],
                                     start=(c == 0), stop=(c == last_c))
            rec = normp.tile([P, T], F32, tag="rec")
            nc.vector.reciprocal(rec, den)
            for dc in range(ndc):
                blk = blks[dc]
                if not is_b:
                    nc.vector.tensor_tensor(outacc[:, blk, :], yts[dc][:, :], rec, mult)
                else:
                    tmp = normp.tile([P, T], F32, tag="btmp")
                    nc.vector.tensor_tensor(tmp, yts[dc][:, :], rec, mult)
                    nc.gpsimd.tensor_tensor(outacc[:, blk, :], outacc[:, blk, :],
                                            tmp, add)
                    nc.sync.dma_start(out=outT[P * blk:P * (blk + 1), :],
                                      in_=outacc[:, blk, :])

        # ================= config A =================
        for h in range(cfg.NA):
            q1 = rawp.tile([P, T], F32, tag="rS")
            nc.sync.dma_start(out=q1, in_=D["qT1"][P * h:P * (h + 1), :])
            q2 = rawp.tile([P, 2, T], F32, tag="rD")
            nc.sync.dma_start(out=q2, in_=D["qT2"][256 * h:256 * (h + 1), :]
                              .rearrange("(c p) t -> p c t", p=P))
            qmix = mixp.tile([P, 2, T], F32, tag="qmix")
            mix_qk_A(qmix, q1, q2, tabs["ca1"], tabs["sa1"], tabs["ca2"], tabs["sa2"])

            k1 = rawp.tile([P, T], F32, tag="rS")
            nc.sync.dma_start(out=k1, in_=D["kTa1"][P * h:P * (h + 1), :])
            k2 = rawp.tile([P, 2, T], F32, tag="rD")
            nc.sync.dma_start(out=k2, in_=D["kTa2"][256 * h:256 * (h + 1), :]
                              .rearrange("(c p) t -> p c t", p=P))
            kmix = mixp.tile([P, 2, T], F32, tag="kmix")
            mix_qk_A(kmix, k1, k2, tabs["ca1"], tabs["sa1"], tabs["ca2"], tabs["sa2"])

            v1 = rawp.tile([P, TK, P], F32, tag="rv1")
            nc.sync.dma_start(out=v1, in_=D["va1"][:, P * h:P * (h + 1)]
                              .rearrange("(c p) d -> p c d", p=P))
            v2 = rawp.tile([P, TK, 2 * P], F32, tag="rv2")
            nc.sync.dma_start(out=v2, in_=D["va2"][:, 2 * P * h:2 * P * (h + 1)]
                              .rearrange("(c p) d -> p c d", p=P))
            vmix = mixp.tile([P, TK, 2 * P], F32, tag="vmix")
            e = pick(2.0, dve_only=True)
            e.tensor_scalar_mul(vmix, v2, wv[:, 1:2])
            e = pick(1.0, dve_only=True)
            e.scalar_tensor_tensor(out=vmix[:, :, 0:P], in0=v1, scalar=wv[:, 0:1],
                                   in1=vmix[:, :, 0:P], op0=mult, op1=add)

            attn_head([qmix[:, 0, :], qmix[:, 1, :]],
                      [kmix[:, 0, :], kmix[:, 1, :]],
                      vmix, (2 * h, 2 * h + 1), is_b=False)

        # ================= config B =================
        for j in range(cfg.NKVB):  # kv head j serves B-heads (2j, 2j+1)
            k2 = rawp.tile([P, T], F32, tag="rS")
            nc.sync.dma_start(out=k2, in_=D["kTb2"][P * j:P * (j + 1), :])
            # packed pair of d=64 kv slices: kv (2*(j//2)), (2*(j//2)+1)
            k1p = rawp.tile([P, T], F32, tag="rS")
            jp = j // 2
            nc.sync.dma_start(out=k1p, in_=D["kTb1"][P * jp:P * (jp + 1), :])

            kmix = mixp.tile([P, T], F32, tag="kmix")
            t1 = scr.tile([P, T], F32, tag="t1")
            pick().tensor_tensor(kmix, k2, tabs["cb2"][:, 0, :], mult)
            halfmul(t1, k2, tabs["sb2"][:, 0, :], 64)
            pick().tensor_tensor(kmix, kmix, t1, add)
            # d=64 part only on rows 0:64 (uses half of the packed pair tile)
            half = 0 if j % 2 == 0 else 64
            sl = slice(half, half + 64)
            ts = scr.tile([P, T], F32, tag="t2", name="ts")
            pick(0.5).tensor_tensor(ts[sl, :], k1p[sl, :],
                                    tabs["cb1"][sl, 0, :], mult)
            tb = scr.tile([P, T], F32, tag="t3", name="tb")
            xb_mul(tb[half:half + 32, :], half, k1p[half + 32:half + 64, :],
                   tabs["sb1"][half:half + 32, 0, :], 0.25)
            xb_mul(tb[half + 32:half + 64, :], half + 32, k1p[half:half + 32, :],
                   tabs["sb1"][half + 32:half + 64, 0, :], 0.25)
            pick(0.5).tensor_tensor(ts[sl, :], ts[sl, :], tb[sl, :], add)
            if half == 0:
                pick(0.5).tensor_tensor(kmix[0:64, :], kmix[0:64, :], ts[sl, :], add)
            else:
                xb_add(kmix[0:64, :], 0, ts[sl, :], 0.5)

            v2 = rawp.tile([P, TK, P], F32, tag="rv1")
            nc.sync.dma_start(out=v2, in_=D["vb2"][:, P * j:P * (j + 1)]
                              .rearrange("(c p) d -> p c d", p=P))
            v1 = rawp.tile([P, TK, 64], F32, tag="rv2")
            nc.sync.dma_start(out=v1, in_=D["vb1"][:, 64 * j:64 * (j + 1)]
                              .rearrange("(c p) d -> p c d", p=P))
            vmix = mixp.tile([P, TK, P], F32, tag="vmix")
            e = pick(1.0, dve_only=True)
            e.tensor_scalar_mul(vmix, v2, wv[:, 3:4])
            e = pick(0.5, dve_only=True)
            e.scalar_tensor_tensor(out=vmix[:, :, 0:64], in0=v1, scalar=wv[:, 2:3],
                                   in1=vmix[:, :, 0:64], op0=mult, op1=add)

            # q pair for heads (2j, 2j+1)
            q2p = rawp.tile([P, 2, T], F32, tag="rD")
            nc.sync.dma_start(out=q2p, in_=D["qT2"][256 * j:256 * (j + 1), :]
                              .rearrange("(c p) t -> p c t", p=P))
            q1p = rawp.tile([P, T], F32, tag="rS")
            nc.sync.dma_start(out=q1p, in_=D["qT1"][P * j:P * (j + 1), :])
            qmixp = mixp.tile([P, 2, T], F32, tag="qmix")
            mix_qk_B_pair(qmixp, q1p, q2p, tabs["cb1"], tabs["sb1"],
                          tabs["cb2"], tabs["sb2"])

            for hh in range(2):
                b = 2 * j + hh
                attn_head([qmixp[:, hh, :]], [kmix], vmix, (b,), is_b=True)

    nc.compile()
    return nc


# ---------------------------------------------------------------------------
# Host side
# ---------------------------------------------------------------------------

def _rope_tab(pos, d, f):
    """Transposed rope tables [d, T]: (f*cos, +-f*sin with rot sign folded)."""
    inv = 1.0 / (10000.0 ** (np.arange(0, d, 2, dtype=np.float32) / d))
    ang = inv[:, None] * pos[None, :].astype(np.float32)      # [d/2, T]
    ang = np.concatenate([ang, ang], 0)                        # [d, T]
    c = (f * np.cos(ang)).astype(np.float32)
    s = (f * np.sin(ang)).astype(np.float32)
    s[: d // 2] *= -1.0
    return c, s


def make_core_inputs(q, k, v, pos, weights, s, cfg: KCfg = FULL):
    """q,k,v: [T, 2048] for one batch; returns the per-core input dict."""
    T = cfg.T
    c = np.ascontiguousarray
    arrs = {
        "qT1": c(q[:, 512 * s:512 * s + 512].T),
        "qT2": c(q[:, 1024 * s:1024 * s + 1024].T),
        "kTa1": c(k[:, 512 * s:512 * s + 512].T),
        "kTa2": c(k[:, 1024 * s:1024 * s + 1024].T),
        "kTb1": c(k[:, 256 * s:256 * s + 256].T),
        "kTb2": c(k[:, 512 * s:512 * s + 512].T),
        "va1": c(v[:, 512 * s:512 * s + 512]),
        "va2": c(v[:, 1024 * s:1024 * s + 1024]),
        "vb1": c(v[:, 256 * s:256 * s + 256]),
        "vb2": c(v[:, 512 * s:512 * s + 512]),
    }
    fA = math.sqrt(1.0 / 16.0)
    fB = math.sqrt(1.0 / math.sqrt(128.0))
    ca1, sa1 = _rope_tab(pos, 128, fA * float(weights[0]))
    ca2, sa2 = _rope_tab(pos, 256, fA * float(weights[1]))
    cb1h, sb1h = _rope_tab(pos, 64, fB * float(weights[2]))
    cb2, sb2 = _rope_tab(pos, 128, fB * float(weights[3]))
    arrs.update({
        "ca1": ca1, "sa1": sa1, "ca2": ca2, "sa2": sa2,
        "cb1": np.vstack([cb1h, cb1h]), "sb1": np.vstack([sb1h, sb1h]),
        "cb2": cb2, "sb2": sb2,
        "wvec": np.tile(np.asarray(weights, np.float32)[None, :], (P, 1)),
    })
    return arrs


_PROGRAM_CACHE = {}
TRACE = False
LAST_RESULT = None


def kernel(q_m, k_m, v_m, weights, attention_mask, position_ids):
    global LAST_RESULT
    from concourse.bass_utils import run_bass_kernel_spmd

    cfg = FULL
    q_m = np.asarray(q_m, np.float32)
    k_m = np.asarray(k_m, np.float32)
    v_m = np.asarray(v_m, np.float32)
    weights = np.asarray(weights, np.float32)
    attention_mask = np.asarray(attention_mask, np.float32)
    position_ids = np.asarray(position_ids)
    B, T, H = q_m.shape

    # the device program hardcodes the causal structure; verify it holds
    causal = np.where(np.tril(np.ones((T, T), bool)), 0.0, NEG).astype(np.float32)
    for b in range(B):
        assert np.array_equal(attention_mask[b, 0], causal), "non-causal mask"

    if "nc" not in _PROGRAM_CACHE:
        _PROGRAM_CACHE["nc"] = build_program(cfg)
    nc = _PROGRAM_CACHE["nc"]

    in_maps = []
    for b in range(B):
        for s in range(2):
            in_maps.append(make_core_inputs(
                q_m[b], k_m[b], v_m[b], position_ids[b], weights, s, cfg))
    res = run_bass_kernel_spmd(nc, in_maps, list(range(8)), trace=TRACE)
    LAST_RESULT = res
    out = np.zeros((B, T, H), np.float32)
    for b in range(B):
        for s in range(2):
            out[b, :, 1024 * s:1024 * s + 1024] = res.results[2 * b + s]["outT"].T
    return out


# revision 41
# speedup vs baseline: 23.1197x; 23.1197x over previous
"""Trainium2 Bass kernel for nn_MixedAttnHeadEmbed (mixed-head-config attention).

Math (per batch b):
  Two attention configs share q_m/k_m/v_m [B,T,2048]:
    A: h=8  heads, d_max=256, mixing e in {1024,2048} -> d in {128,256}, weights w0,w1
    B: h=16 heads, d_max=128, mixing e in {1024,2048} -> d in {64,128},  weights w2,w3
  Each config: per-head q/k slices are RoPE'd, weight-summed (padded to d_max),
  GQA (8 kv heads), causal softmax attention; outputs of both configs sum.

Sharding: 8 cores = 4 batches x 2 shards. Shard s owns A-heads [4s,4s+4) and
B-heads [8s,8s+8) -> both write output columns [1024s, 1024s+1024) which are
summed on device; per-core output is the transposed block outT [1024, T].

Device layout: scores computed transposed (sT[k,q], k on partitions) so the
softmax'd weights feed the y^T matmul with no on-chip transposes; softmax is
max-free (scores are provably < 2 for this problem family; exp is safe in
fp32) with the denominator from an all-ones stationary matmul.
"""

import math
from contextlib import ExitStack
from dataclasses import dataclass

import numpy as np

import concourse.bass as bass
import concourse.mybir as mybir
import concourse.tile as tile
from concourse import bacc

F32 = mybir.dt.float32
F32R = mybir.dt.float32r
NEG = -1e9
P = 128


@dataclass(frozen=True)
class KCfg:
    T: int = 1024       # sequence length
    NA: int = 4         # config-A heads per core (d_max=256)
    NB: int = 8         # config-B heads per core (d_max=128); must be 2*NA
    REG: int = 512      # psum region width (<=512)

    @property
    def TK(self):
        return self.T // P

    @property
    def NREG(self):
        return self.T // self.REG

    @property
    def NKVB(self):
        return self.NB // 2

    @property
    def ROWS(self):
        return self.NA * 256  # == NB * 128 output rows per core


FULL = KCfg()


def _in_specs(cfg: KCfg):
    T = cfg.T
    return {
        "qT1": (cfg.NA * 128, T),
        "qT2": (cfg.NA * 256, T),
        "kTa1": (cfg.NA * 128, T),
        "kTa2": (cfg.NA * 256, T),
        "kTb1": (cfg.NKVB * 64, T),
        "kTb2": (cfg.NKVB * 128, T),
        "va1": (T, cfg.NA * 128),
        "va2": (T, cfg.NA * 256),
        "vb1": (T, cfg.NKVB * 64),
        "vb2": (T, cfg.NKVB * 128),
        "ca1": (128, T), "sa1": (128, T),
        "ca2": (256, T), "sa2": (256, T),
        "cb1": (128, T), "sb1": (128, T),
        "cb2": (128, T), "sb2": (128, T),
        "wvec": (P, 4),
    }


class _EngPick:
    """Static load balancer across DVE / GPSIMD / ACT.

    units: 1.0 ~ one [.,1024] fp32 pass. Cost-model calibration: DVE and
    Pool run TT at ~1 elem/lane/cycle (fp32 has no DVE fast mode); ACT can
    only take single-input copies, and it also carries all the exps (those
    are tallied in via act())."""

    GP_W = 1.05   # tuned: bias work toward pool
    ACT_W = 1.5

    def __init__(self, nc):
        self.nc = nc
        self.load = {"dve": 0.0, "pool": 0.0, "act": 0.0}

    def dve(self, units=1.0):
        self.load["dve"] += units
        return self.nc.vector

    def act(self, units=1.0):
        self.load["act"] += units * self.ACT_W
        return self.nc.scalar

    def tt(self, units=1.0):
        """2-input sbuf op: DVE or GPSIMD."""
        if self.load["dve"] + units <= self.load["pool"] + self.GP_W * units:
            return self.dve(units)
        self.load["pool"] += self.GP_W * units
        return self.nc.gpsimd

    def copy(self, dst, src, units=1.0):
        """1-input copy: any of the three engines."""
        costs = {"dve": units, "pool": self.GP_W * units,
                 "act": self.ACT_W * units}
        eng = min(costs, key=lambda k: self.load[k] + costs[k])
        self.load[eng] += costs[eng]
        if eng == "act":
            self.nc.scalar.copy(dst, src)
        elif eng == "pool":
            self.nc.gpsimd.tensor_copy(dst, src)
        else:
            self.nc.vector.tensor_copy(dst, src)


def build_program(cfg: KCfg = FULL):
    # Bacc (not plain Bass): its compile() runs generate_event_semaphores,
    # which splits multi-wait sync_infos — TRN2 allows 1 wait per instruction.
    nc = bacc.Bacc("TRN2", target_bir_lowering=False)
    T, TK, REG, NREG = cfg.T, cfg.TK, cfg.REG, cfg.NREG
    RPB = REG // P  # k-chunks per region

    D = {}
    for name, shape in _in_specs(cfg).items():
        D[name] = nc.declare_dram_parameter(name, list(shape), F32, isOutput=False)
    outT = nc.declare_dram_parameter("outT", [cfg.ROWS, T], F32, isOutput=True)
    RB = cfg.ROWS // P

    mult, add = mybir.AluOpType.mult, mybir.AluOpType.add

    with ExitStack() as ctx:
        tc = ctx.enter_context(tile.TileContext(nc))
        const = ctx.enter_context(tc.tile_pool(name="const", bufs=1))
        rawp = ctx.enter_context(tc.tile_pool(name="raw", bufs=2))
        mixp = ctx.enter_context(tc.tile_pool(name="mix", bufs=2))
        scr = ctx.enter_context(tc.tile_pool(name="scr", bufs=1))
        ppool = ctx.enter_context(tc.tile_pool(name="pp", bufs=3))
        accp = ctx.enter_context(tc.tile_pool(name="acc", bufs=1))
        normp = ctx.enter_context(tc.tile_pool(name="norm", bufs=1))
        spsum = ctx.enter_context(tc.tile_pool(name="spsum", bufs=2, space="PSUM"))
        ypsum = ctx.enter_context(tc.tile_pool(name="ypsum", bufs=1, space="PSUM"))
        dpsum = ctx.enter_context(tc.tile_pool(name="dpsum", bufs=1, space="PSUM"))

        pick = _EngPick(nc)

        # ---- constants ----
        ones_f = const.tile([P, P], F32, name="ones_f")
        nc.vector.memset(ones_f, 1.0)
        ones = const.tile([P, P], F32R)
        nc.vector.tensor_copy(ones, ones_f)  # rounds to f32r for the matmul
        dmask = const.tile([P, P], F32)
        nc.gpsimd.memset(dmask, 0.0)
        # dmask[k, q] = 0 where q >= k else NEG  (transposed causal diag block)
        nc.gpsimd.affine_select(
            out=dmask, in_=dmask, compare_op=mybir.AluOpType.is_ge,
            fill=NEG, base=0, pattern=[[1, P]], channel_multiplier=-1,
        )
        tabs = {}
        for nm in ("ca1", "sa1", "ca2", "sa2", "cb1", "sb1", "cb2", "sb2"):
            rows = _in_specs(cfg)[nm][0]
            tl = const.tile([P, rows // P, T], F32, name=nm, tag=nm)
            tabs[nm] = tl
            nc.sync.dma_start(out=tl, in_=D[nm].rearrange("(c p) t -> p c t", p=P))
        wv = const.tile([P, 4], F32)
        nc.sync.dma_start(out=wv, in_=D["wvec"][:, :])

        outacc = accp.tile([P, RB, T], F32)

        def halfmul(dst, src, tab, half, base=0, rows=P):
            """dst[base:base+rows][j] = src[sigma(j)] * tab_math[j], where
            sigma swaps halves of size `half` within each 2*half group.

            tab is the HOST-SIGMA-PERMUTED signed sin table, so the multiply
            is same-base (u = src*tab) and the rotation becomes 1-input
            cross-base copies (the only cross-partition-base op trn2 allows).
            """
            u = scr.tile([P, T], F32, tag="xbt", name="xbt")
            usl = u[base:base + rows, :]
            pick.tt(1.0).tensor_tensor(usl, src, tab, mult)
            for g in range(rows // (2 * half)):
                b0 = base + 2 * half * g
                pick.copy(dst[b0:b0 + half, :], u[b0 + half:b0 + 2 * half, :], 1.0)
                pick.copy(dst[b0 + half:b0 + 2 * half, :], u[b0:b0 + half, :], 1.0)

        def xb_add(dst, src, units):
            """dst += src across partition bases (copy to re-base, then add)."""
            n = src.shape[0]
            tmp = scr.tile([P, T], F32, tag="xbt2", name="xbt2")
            view = tmp[0:n, :]
            pick.copy(view, src, units)
            pick.tt(units).tensor_tensor(dst, dst, view, add)

        def mix_qk_A(out, x1, x2, c1, s1, c2, s2):
            """out [P,2,T] = rope-mix for a config-A head.
            x1 [P,T] (d=128 slice), x2 [P,2,T] (d=256 slice).
            s1 is sigma64-permuted; s2 is the plain signed sin table."""
            t1 = scr.tile([P, T], F32, tag="t1")
            t2 = scr.tile([P, T], F32, tag="t2")
            # dc0: x2t0*c2_0 + x2t1*s2_0 + x1*c1 + shift64(x1)*s1
            pick.tt().tensor_tensor(out[:, 0, :], x2[:, 0, :], c2[:, 0, :], mult)
            pick.tt().tensor_tensor(t1, x2[:, 1, :], s2[:, 0, :], mult)
            pick.tt().tensor_tensor(out[:, 0, :], out[:, 0, :], t1, add)
            pick.tt().tensor_tensor(t1, x1, c1[:, 0, :], mult)
            halfmul(t2, x1, s1[:, 0, :], 64)
            pick.tt().tensor_tensor(t1, t1, t2, add)
            pick.tt().tensor_tensor(out[:, 0, :], out[:, 0, :], t1, add)
            # dc1: x2t1*c2_1 + x2t0*s2_1
            pick.tt().tensor_tensor(out[:, 1, :], x2[:, 1, :], c2[:, 1, :], mult)
            pick.tt().tensor_tensor(t1, x2[:, 0, :], s2[:, 1, :], mult)
            pick.tt().tensor_tensor(out[:, 1, :], out[:, 1, :], t1, add)

        def mix_qk_B_pair(out, x1p, x2p, c1, s1, c2, s2):
            """out [P,2,T]: B-head pair. out[:,j,:] for heads (2p+j).
            x2p [P,2,T] (d=128 per head), x1p [P,T] packed pair (d=64 each).
            s2 sigma64-permuted; s1 sigma32-permuted."""
            t1 = scr.tile([P, T], F32, tag="t1")
            t2 = scr.tile([P, T], F32, tag="t2")
            for j in range(2):
                pick.tt().tensor_tensor(out[:, j, :], x2p[:, j, :], c2[:, 0, :], mult)
                halfmul(t1, x2p[:, j, :], s2[:, 0, :], 64)
                pick.tt().tensor_tensor(out[:, j, :], out[:, j, :], t1, add)
            # packed d=64 contributions for both heads of the pair
            pick.tt().tensor_tensor(t1, x1p, c1[:, 0, :], mult)
            halfmul(t2, x1p, s1[:, 0, :], 32)
            pick.tt().tensor_tensor(t1, t1, t2, add)
            pick.tt(1.0).tensor_tensor(out[0:64, 0, :], out[0:64, 0, :],
                                       t1[0:64, :], add)
            xb_add(out[0:64, 1, :], t1[64:128, :], 1.0)

        def subchunks(c):
            out = []
            for r in range(NREG):
                q0 = max(REG * r, P * c)
                q1 = REG * (r + 1)
                if q1 > q0:
                    out.append((r, q0, q1 - q0))
            return out

        def attn_head(qmixs, kmixs, vmix, blks, is_b):
            """qmixs/kmixs: per-d-chunk [P, T] APs; vmix [P, TK, ndc*P].

            Matmul operands are bitcast to float32r: full-rate PE streaming
            (fp32 proper runs at 1/4 rate) with near-fp32 accumulation."""
            ndc = len(qmixs)
            den = dpsum.tile([P, T], F32, tag="den")
            yts = [ypsum.tile([P, T], F32, tag=f"yt{i}", name=f"yt{i}")
                   for i in range(ndc)]
            for c in range(TK):
                for (r, q0, n) in subchunks(c):
                    last_c = min(TK, RPB * (r + 1)) - 1
                    sT = spsum.tile([P, REG], F32, tag="sT")
                    for dc in range(ndc):
                        nc.tensor.matmul(
                            sT[:, :n],
                            kmixs[dc][:, P * c:P * (c + 1)],
                            qmixs[dc][:, q0:q0 + n],
                            start=(dc == 0), stop=(dc == ndc - 1))
                    if q0 == P * c:  # diagonal block gets the causal mask
                        pick.dve(0.125).tensor_tensor(sT[:, :P], sT[:, :P],
                                                      dmask, add)
                    pt = ppool.tile([P, REG], F32R, tag="pT")
                    pick.act(n / 1024.0).activation(
                        pt[:, :n], sT[:, :n], mybir.ActivationFunctionType.Exp)
                    for dc in range(ndc):
                        nc.tensor.matmul(
                            yts[dc][:, q0:q0 + n],
                            vmix[:, c, P * dc:P * (dc + 1)],
                            pt[:, :n],
                            start=(c == 0), stop=(c == last_c))
                    nc.tensor.matmul(den[:, q0:q0 + n], ones,
                                     pt[:, :n],
                                     start=(c == 0), stop=(c == last_c))
            rec = normp.tile([P, T], F32, tag="rec")
            pick.dve(1.0).reciprocal(rec, den)
            for dc in range(ndc):
                blk = blks[dc]
                if not is_b:
                    pick.dve(1.0).tensor_tensor(outacc[:, blk, :], yts[dc][:, :],
                                                rec, mult)
                else:
                    tmp = normp.tile([P, T], F32, tag="btmp")
                    pick.dve(1.0).tensor_tensor(tmp, yts[dc][:, :], rec, mult)
                    pick.tt(1.0).tensor_tensor(outacc[:, blk, :],
                                               outacc[:, blk, :], tmp, add)
                    nc.sync.dma_start(out=outT[P * blk:P * (blk + 1), :],
                                      in_=outacc[:, blk, :])

        # ================= config A =================
        for h in range(cfg.NA):
            q1 = rawp.tile([P, T], F32, tag="rS")
            nc.sync.dma_start(out=q1, in_=D["qT1"][P * h:P * (h + 1), :])
            q2 = rawp.tile([P, 2, T], F32, tag="rD")
            nc.sync.dma_start(out=q2, in_=D["qT2"][256 * h:256 * (h + 1), :]
                              .rearrange("(c p) t -> p c t", p=P))
            qmix = mixp.tile([P, 2, T], F32R, tag="qmix")
            mix_qk_A(qmix, q1, q2, tabs["ca1"], tabs["sa1"], tabs["ca2"], tabs["sa2"])

            k1 = rawp.tile([P, T], F32, tag="rS")
            nc.sync.dma_start(out=k1, in_=D["kTa1"][P * h:P * (h + 1), :])
            k2 = rawp.tile([P, 2, T], F32, tag="rD")
            nc.sync.dma_start(out=k2, in_=D["kTa2"][256 * h:256 * (h + 1), :]
                              .rearrange("(c p) t -> p c t", p=P))
            kmix = mixp.tile([P, 2, T], F32R, tag="kmix")
            mix_qk_A(kmix, k1, k2, tabs["ca1"], tabs["sa1"], tabs["ca2"], tabs["sa2"])

            v1 = rawp.tile([P, TK, P], F32, tag="rv1")
            nc.sync.dma_start(out=v1, in_=D["va1"][:, P * h:P * (h + 1)]
                              .rearrange("(c p) d -> p c d", p=P))
            v2 = rawp.tile([P, TK, 2 * P], F32, tag="rv2")
            nc.sync.dma_start(out=v2, in_=D["va2"][:, 2 * P * h:2 * P * (h + 1)]
                              .rearrange("(c p) d -> p c d", p=P))
            vmix = mixp.tile([P, TK, 2 * P], F32R, tag="vmix")
            pick.dve(2.0).tensor_scalar_mul(vmix, v2, wv[:, 1:2])
            pick.dve(1.0).scalar_tensor_tensor(
                out=vmix[:, :, 0:P], in0=v1, scalar=wv[:, 0:1],
                in1=vmix[:, :, 0:P], op0=mult, op1=add)

            attn_head([qmix[:, 0, :], qmix[:, 1, :]],
                      [kmix[:, 0, :], kmix[:, 1, :]],
                      vmix, (2 * h, 2 * h + 1), is_b=False)

        # ================= config B =================
        for j in range(cfg.NKVB):  # kv head j serves B-heads (2j, 2j+1)
            k2 = rawp.tile([P, T], F32, tag="rS")
            nc.sync.dma_start(out=k2, in_=D["kTb2"][P * j:P * (j + 1), :])
            # packed pair of d=64 kv slices: kv (2*(j//2)), (2*(j//2)+1)
            k1p = rawp.tile([P, T], F32, tag="rS")
            jp = j // 2
            nc.sync.dma_start(out=k1p, in_=D["kTb1"][P * jp:P * (jp + 1), :])

            kmix = mixp.tile([P, T], F32R, tag="kmix")
            t1 = scr.tile([P, T], F32, tag="t1")
            pick.tt().tensor_tensor(kmix, k2, tabs["cb2"][:, 0, :], mult)
            halfmul(t1, k2, tabs["sb2"][:, 0, :], 64)
            pick.tt().tensor_tensor(kmix, kmix, t1, add)
            # d=64 part only on rows 0:64 (uses half of the packed pair tile)
            half = 0 if j % 2 == 0 else 64
            sl = slice(half, half + 64)
            ts = scr.tile([P, T], F32, tag="t2", name="ts")
            pick.tt().tensor_tensor(ts[sl, :], k1p[sl, :],
                                    tabs["cb1"][sl, 0, :], mult)
            tb = scr.tile([P, T], F32, tag="t3", name="tb")
            halfmul(tb, k1p[sl, :], tabs["sb1"][sl, 0, :], 32, base=half, rows=64)
            pick.tt().tensor_tensor(ts[sl, :], ts[sl, :], tb[sl, :], add)
            if half == 0:
                pick.tt().tensor_tensor(kmix[0:64, :], kmix[0:64, :], ts[sl, :], add)
            else:
                xb_add(kmix[0:64, :], ts[sl, :], 1.0)

            v2 = rawp.tile([P, TK, P], F32, tag="rv1")
            nc.sync.dma_start(out=v2, in_=D["vb2"][:, P * j:P * (j + 1)]
                              .rearrange("(c p) d -> p c d", p=P))
            v1 = rawp.tile([P, TK, 64], F32, tag="rv2")
            nc.sync.dma_start(out=v1, in_=D["vb1"][:, 64 * j:64 * (j + 1)]
                              .rearrange("(c p) d -> p c d", p=P))
            vmix = mixp.tile([P, TK, P], F32R, tag="vmix")
            pick.dve(1.0).tensor_scalar_mul(vmix, v2, wv[:, 3:4])
            pick.dve(0.5).scalar_tensor_tensor(
                out=vmix[:, :, 0:64], in0=v1, scalar=wv[:, 2:3],
                in1=vmix[:, :, 0:64], op0=mult, op1=add)

            # q pair for heads (2j, 2j+1)
            q2p = rawp.tile([P, 2, T], F32, tag="rD")
            nc.sync.dma_start(out=q2p, in_=D["qT2"][256 * j:256 * (j + 1), :]
                              .rearrange("(c p) t -> p c t", p=P))
            q1p = rawp.tile([P, T], F32, tag="rS")
            nc.sync.dma_start(out=q1p, in_=D["qT1"][P * j:P * (j + 1), :])
            qmixp = mixp.tile([P, 2, T], F32R, tag="qmix")
            mix_qk_B_pair(qmixp, q1p, q2p, tabs["cb1"], tabs["sb1"],
                          tabs["cb2"], tabs["sb2"])

            for hh in range(2):
                b = 2 * j + hh
                attn_head([qmixp[:, hh, :]], [kmix], vmix, (b,), is_b=True)

    nc.compile()
    return nc


# ---------------------------------------------------------------------------
# Host side
# ---------------------------------------------------------------------------

def _rope_tab(pos, d, f):
    """Transposed rope tables [d, T]: (f*cos, +-f*sin with rot sign folded)."""
    inv = 1.0 / (10000.0 ** (np.arange(0, d, 2, dtype=np.float32) / d))
    ang = inv[:, None] * pos[None, :].astype(np.float32)      # [d/2, T]
    ang = np.concatenate([ang, ang], 0)                        # [d, T]
    c = (f * np.cos(ang)).astype(np.float32)
    s = (f * np.sin(ang)).astype(np.float32)
    s[: d // 2] *= -1.0
    return c, s


def make_core_inputs(q, k, v, pos, weights, s, cfg: KCfg = FULL):
    """q,k,v: [T, 2048] for one batch; returns the per-core input dict."""
    T = cfg.T
    c = np.ascontiguousarray
    arrs = {
        "qT1": c(q[:, 512 * s:512 * s + 512].T),
        "qT2": c(q[:, 1024 * s:1024 * s + 1024].T),
        "kTa1": c(k[:, 512 * s:512 * s + 512].T),
        "kTa2": c(k[:, 1024 * s:1024 * s + 1024].T),
        "kTb1": c(k[:, 256 * s:256 * s + 256].T),
        "kTb2": c(k[:, 512 * s:512 * s + 512].T),
        "va1": c(v[:, 512 * s:512 * s + 512]),
        "va2": c(v[:, 1024 * s:1024 * s + 1024]),
        "vb1": c(v[:, 256 * s:256 * s + 256]),
        "vb2": c(v[:, 512 * s:512 * s + 512]),
    }
    fA = math.sqrt(1.0 / 16.0)
    fB = math.sqrt(1.0 / math.sqrt(128.0))
    ca1, sa1 = _rope_tab(pos, 128, fA * float(weights[0]))
    ca2, sa2 = _rope_tab(pos, 256, fA * float(weights[1]))
    cb1h, sb1h = _rope_tab(pos, 64, fB * float(weights[2]))
    cb2, sb2 = _rope_tab(pos, 128, fB * float(weights[3]))

    def sigma(tab, half):
        # swap halves of size `half` within each 2*half row group
        out = tab.reshape(-1, 2, half, tab.shape[-1])
        return np.ascontiguousarray(
            out[:, ::-1].reshape(tab.shape))

    sb1 = np.vstack([sb1h, sb1h])
    arrs.update({
        # sin tables used through within-tile rotations are stored
        # sigma-permuted (device computes u = x * s_sigma, then rotates u
        # via cross-base copies); sa2 (d=256) rotates across tiles and
        # stays in math order.
        "ca1": ca1, "sa1": sigma(sa1, 64), "ca2": ca2, "sa2": sa2,
        "cb1": np.vstack([cb1h, cb1h]), "sb1": sigma(sb1, 32),
        "cb2": cb2, "sb2": sigma(sb2, 64),
        "wvec": np.tile(np.asarray(weights, np.float32)[None, :], (P, 1)),
        # math-order copies for numpy models (not used by the device)
        "_m_sa1": sa1, "_m_sb1": sb1, "_m_sb2": sb2,
    })
    return arrs


_PROGRAM_CACHE = {}
TRACE = False
LAST_RESULT = None


def kernel(q_m, k_m, v_m, weights, attention_mask, position_ids):
    global LAST_RESULT
    from concourse.bass_utils import run_bass_kernel_spmd

    cfg = FULL
    q_m = np.asarray(q_m, np.float32)
    k_m = np.asarray(k_m, np.float32)
    v_m = np.asarray(v_m, np.float32)
    weights = np.asarray(weights, np.float32)
    attention_mask = np.asarray(attention_mask, np.float32)
    position_ids = np.asarray(position_ids)
    B, T, H = q_m.shape

    # the device program hardcodes the causal structure; verify it holds
    causal = np.where(np.tril(np.ones((T, T), bool)), 0.0, NEG).astype(np.float32)
    for b in range(B):
        assert np.array_equal(attention_mask[b, 0], causal), "non-causal mask"

    if "nc" not in _PROGRAM_CACHE:
        _PROGRAM_CACHE["nc"] = build_program(cfg)
    nc = _PROGRAM_CACHE["nc"]

    in_maps = []
    for b in range(B):
        for s in range(2):
            in_maps.append(make_core_inputs(
                q_m[b], k_m[b], v_m[b], position_ids[b], weights, s, cfg))
    res = run_bass_kernel_spmd(nc, in_maps, list(range(8)), trace=TRACE)
    LAST_RESULT = res
    out = np.zeros((B, T, H), np.float32)
    for b in range(B):
        for s in range(2):
            out[b, :, 1024 * s:1024 * s + 1024] = res.results[2 * b + s]["outT"].T
    return out
